# revision 6
# baseline (speedup 1.0000x reference)
"""Causal multi-head attention block (qkv proj + attention + out proj) on 8
Trainium2 NeuronCores.

Sharding: core c = 2*b + hg handles batch b (of 4) and head-group hg (8 of 16
heads).  Each core computes qkv for its heads, causal attention, and a partial
out-projection (its 512 rows of w_out); the host sums the two head-group
partials per batch.

Per-core kernel layout (all matmuls fp32r = full-rate TF32-ish):
  - x [T, DIM] is PE-transposed into xT [DIM, T] (contraction over DIM needs
    dim on partitions for both qkv operands).
  - Q^T, K^T stored as 4 head-pair tiles [128 = 2 heads x 64, T]; scores are
    computed transposed: S^T[k, q] = K^T.T-free... lhsT=K^T tile, rhs=Q^T tile,
    two heads concurrently via PE row tiling (K=64 each).
  - softmax runs without max-subtraction (scores ~ N(0,1), exp is safe in
    fp32); denominator comes free from a ones-column appended to V
    (V_aug [k, 8*65]): P @ V_aug accumulates numerator and row-sum together.
  - causal mask = 0/1 multiply on diagonal 128x512 blocks only (DVE); the
    padding mask multiplies V_aug rows (exactly equivalent to masking P).
  - normalize: DVE reciprocal of the sum row, PE rank-1 broadcast, DVE mul.
"""

import sys

if "/opt/trn_rl_repo" not in sys.path:
    sys.path.insert(0, "/opt/trn_rl_repo")

import numpy as np

import concourse.bass as bass
import concourse.mybir as mybir
import concourse.tile as tile
from concourse import bacc
from concourse.bass_utils import run_bass_kernel_spmd
from concourse.masks import make_identity

DIM = 1024
N_HEAD = 16
HD = 64
B, T = 4, 2048
HG = 8          # heads per core
CQ = HG * HD    # 512 feature columns per group
NCORES = 8

f32 = mybir.dt.float32
f32r = mybir.dt.float32r

Exp = mybir.ActivationFunctionType.Exp


def build_nc():
    nc = bacc.Bacc(None, target_bir_lowering=False)
    x_d = nc.declare_dram_parameter("x", [T, DIM], f32, isOutput=False)
    wqk_d = nc.declare_dram_parameter("wqk", [DIM, 2 * CQ], f32, isOutput=False)
    wv_d = nc.declare_dram_parameter("wv", [DIM, CQ], f32, isOutput=False)
    wo_d = nc.declare_dram_parameter("wo", [CQ, DIM], f32, isOutput=False)
    mv_d = nc.declare_dram_parameter("maskv", [128, T // 128], f32, isOutput=False)
    out_d = nc.declare_dram_parameter("out", [T, DIM], f32, isOutput=True)

    NT = T // 128          # 16 t-subtiles
    NQ = T // 512          # 4 q/quarter blocks

    with tile.TileContext(nc) as tc:
        with tc.tile_pool(name="persist", bufs=1) as pp:
            ident32 = pp.tile([128, 128], f32, name="ident32", tag="ident32")
            make_identity(nc, ident32)
            ident = pp.tile([128, 128], f32r, name="ident", tag="ident")
            nc.vector.tensor_copy(ident, ident32)
            ones32 = pp.tile([1, 64], f32, name="ones32", tag="ones32")
            nc.vector.memset(ones32, 1.0)
            ones64 = pp.tile([1, 64], f32r, name="ones64", tag="ones64")
            nc.vector.tensor_copy(ones64, ones32)
            onescol = pp.tile([128, HG], f32, name="onescol", tag="onescol")
            nc.vector.memset(onescol, 1.0)
            # diag[j]: keep (1.0) where q_local - k_local - 128*j >= 0
            diag = []
            for j in range(4):
                d = pp.tile([128, 512], f32, name=f"diag{j}", tag=f"diag{j}")
                nc.gpsimd.memset(d, 1.0)
                nc.gpsimd.affine_select(
                    out=d, in_=d,
                    compare_op=mybir.AluOpType.is_ge,
                    fill=0.0, base=-128 * j,
                    pattern=[[1, 512]], channel_multiplier=-1,
                )
                diag.append(d)
            mv_sb = pp.tile([128, NT], f32, name="maskv_sb", tag="maskv_sb")
            nc.sync.dma_start(out=mv_sb, in_=mv_d[:, :])

            qt = [pp.tile([128, T], f32r, name=f"qt{m}", tag=f"qt{m}") for m in range(4)]
            kt = [pp.tile([128, T], f32r, name=f"kt{m}", tag=f"kt{m}") for m in range(4)]
            vaug = [pp.tile([128, HG * 65], f32r, name=f"vaug{t}", tag=f"vaug{t}")
                    for t in range(NT)]
            wo_sb = [pp.tile([128, DIM], f32r, name=f"wo{m}", tag=f"wo{m}") for m in range(4)]
            for m in range(4):
                nc.sync.dma_start(out=wo_sb[m],
                                  in_=wo_d[m * 128:(m + 1) * 128, :].bitcast(f32r))

            # ---------------- qkv phase (per t-quarter) ----------------
            with tc.tile_pool(name="wqkp", bufs=1) as wqkp, \
                 tc.tile_pool(name="wvp", bufs=1) as wvp, \
                 tc.tile_pool(name="xrow", bufs=1) as xrow_p, \
                 tc.tile_pool(name="xT", bufs=1) as xT_p, \
                 tc.tile_pool(name="ps_t", bufs=4, space="PSUM") as ps_t, \
                 tc.tile_pool(name="ps_mm", bufs=2, space="PSUM") as ps_mm:

                wqk_sb = [wqkp.tile([128, 2 * CQ], f32r, name=f"wqk{k}", tag=f"wqk{k}")
                          for k in range(8)]
                wv_sb = [wvp.tile([128, CQ], f32r, name=f"wv{k}", tag=f"wv{k}")
                         for k in range(8)]
                for k in range(8):
                    nc.sync.dma_start(out=wqk_sb[k],
                                      in_=wqk_d[k * 128:(k + 1) * 128, :].bitcast(f32r))
                    nc.sync.dma_start(out=wv_sb[k],
                                      in_=wv_d[k * 128:(k + 1) * 128, :].bitcast(f32r))

                for q in range(NQ):
                    xrs = []
                    for ti in range(4):
                        xr = xrow_p.tile([128, DIM], f32r, name=f"xr{ti}", tag=f"xr{ti}")
                        t0 = (q * 4 + ti) * 128
                        nc.sync.dma_start(out=xr, in_=x_d[t0:t0 + 128, :].bitcast(f32r))
                        xrs.append(xr)
                    xts = [xT_p.tile([128, 512], f32r, name=f"xt{kb}", tag=f"xt{kb}")
                           for kb in range(8)]
                    for kb in range(8):
                        for ti in range(4):
                            pst = ps_t.tile([128, 128], f32r, name="pst", tag="pst")
                            nc.tensor.transpose(
                                pst, xrs[ti][:, kb * 128:(kb + 1) * 128], ident)
                            nc.vector.tensor_copy(
                                xts[kb][:, ti * 128:(ti + 1) * 128], pst)
                    # Q and K projections: 8 M-tiles (4 q-pairs then 4 k-pairs)
                    for m in range(8):
                        pq = ps_mm.tile([128, 512], f32, name="mm", tag="mm")
                        for kb in range(8):
                            nc.tensor.matmul(
                                pq, wqk_sb[kb][:, m * 128:(m + 1) * 128], xts[kb],
                                start=(kb == 0), stop=(kb == 7))
                        dst = qt[m] if m < 4 else kt[m - 4]
                        nc.vector.tensor_copy(dst[:, q * 512:(q + 1) * 512], pq)
                    # V projection (natural layout) + ones column + padding mask
                    for ti in range(4):
                        pv = ps_mm.tile([128, 512], f32, name="mm", tag="mm")
                        for kb in range(8):
                            nc.tensor.matmul(
                                pv, xts[kb][:, ti * 128:(ti + 1) * 128], wv_sb[kb],
                                start=(kb == 0), stop=(kb == 7))
                        vt = vaug[q * 4 + ti]
                        vt3 = vt.rearrange("p (h w) -> p h w", w=65)
                        pv3 = pv.rearrange("p (h w) -> p h w", w=64)
                        nc.vector.tensor_copy(vt3[:, :, 0:64], pv3)
                        nc.vector.tensor_copy(
                            vt3[:, :, 64:65],
                            onescol.rearrange("p (h w) -> p h w", w=1))
                        nc.vector.tensor_scalar_mul(
                            vt, vt, mv_sb[:, (q * 4 + ti):(q * 4 + ti + 1)])

            # ---------------- attention + out_proj (per q-block) ----------------
            with tc.tile_pool(name="ps_s", bufs=4, space="PSUM") as ps_s, \
                 tc.tile_pool(name="ps_pv", bufs=2, space="PSUM") as ps_pv, \
                 tc.tile_pool(name="ps_bc", bufs=1, space="PSUM") as ps_bc, \
                 tc.tile_pool(name="p_p", bufs=4) as p_p, \
                 tc.tile_pool(name="at_p", bufs=2) as at_p, \
                 tc.tile_pool(name="rec_p", bufs=2) as rec_p, \
                 tc.tile_pool(name="bcs_p", bufs=2) as bcs_p, \
                 tc.tile_pool(name="out_p", bufs=3) as out_p:

                for qb in range(NQ):
                    ats = [at_p.tile([128, 512], f32r, name=f"at{m}", tag=f"at{m}")
                           for m in range(4)]
                    for m in range(4):
                        pve = ps_pv.tile([65, 512], f32, name="pv", tag="pv")
                        pvo = ps_pv.tile([65, 512], f32, name="pv", tag="pv")
                        nk = 4 * (qb + 1)
                        for kti in range(nk):
                            se = ps_s.tile([128, 512], f32, name="s", tag="s")
                            so = ps_s.tile([128, 512], f32, name="s", tag="s")
                            nc.tensor.matmul(
                                se, kt[m][0:64, kti * 128:(kti + 1) * 128],
                                qt[m][0:64, qb * 512:(qb + 1) * 512],
                                start=True, stop=True)
                            nc.tensor.matmul(
                                so, kt[m][64:128, kti * 128:(kti + 1) * 128],
                                qt[m][64:128, qb * 512:(qb + 1) * 512],
                                start=True, stop=True)
                            pe_ = p_p.tile([128, 512], f32r, name="p", tag="p")
                            po_ = p_p.tile([128, 512], f32r, name="p", tag="p")
                            nc.scalar.activation(pe_, se, Exp, scale=0.125)
                            nc.scalar.activation(po_, so, Exp, scale=0.125)
                            j = kti - 4 * qb
                            if j >= 0:  # diagonal block: apply causal 0/1 mask
                                nc.vector.tensor_mul(pe_, pe_, diag[j])
                                nc.vector.tensor_mul(po_, po_, diag[j])
                            h0, h1 = 2 * m, 2 * m + 1
                            nc.tensor.matmul(
                                pve, vaug[kti][:, h0 * 65:(h0 + 1) * 65], pe_,
                                start=(kti == 0), stop=(kti == nk - 1))
                            nc.tensor.matmul(
                                pvo, vaug[kti][:, h1 * 65:(h1 + 1) * 65], po_,
                                start=(kti == 0), stop=(kti == nk - 1))
                        for pv_, half in ((pve, 0), (pvo, 1)):
                            rec = rec_p.tile([1, 512], f32r, name="rec", tag="rec")
                            with nc.allow_low_precision(
                                    reason="fp32r reciprocal feeds fp32r bcast matmul"):
                                nc.vector.reciprocal(rec, pv_[64:65, :])
                            bc = ps_bc.tile([64, 512], f32, name="bc", tag="bc")
                            nc.tensor.matmul(bc, ones64, rec, start=True, stop=True)
                            bcs = bcs_p.tile([64, 512], f32, name="bcs", tag="bcs")
                            nc.scalar.copy(bcs, bc)
                            nc.vector.tensor_mul(
                                ats[m][half * 64:(half + 1) * 64, :],
                                pv_[0:64, :], bcs)
                    # out projection for this q-block
                    for ti in range(4):
                        for nb in range(2):
                            po = ps_s.tile([128, 512], f32, name="s", tag="s")
                            for m in range(4):
                                nc.tensor.matmul(
                                    po, ats[m][:, ti * 128:(ti + 1) * 128],
                                    wo_sb[m][:, nb * 512:(nb + 1) * 512],
                                    start=(m == 0), stop=(m == 3))
                            ob = out_p.tile([128, 512], f32, name="ob", tag="ob")
                            nc.vector.tensor_copy(ob, po)
                            t0 = (qb * 4 + ti) * 128
                            nc.sync.dma_start(
                                out=out_d[t0:t0 + 128, nb * 512:(nb + 1) * 512],
                                in_=ob)
    nc.finalize()
    return nc


_NC_CACHE = {}


def _get_nc():
    if "nc" not in _NC_CACHE:
        _NC_CACHE["nc"] = build_nc()
    return _NC_CACHE["nc"]


def _make_in_maps(x, w_qkv, w_out, attn_mask):
    x = np.asarray(x, dtype=np.float32)
    w_qkv = np.asarray(w_qkv, dtype=np.float32)
    w_out = np.asarray(w_out, dtype=np.float32)
    am = np.asarray(attn_mask)
    in_maps = []
    for c in range(NCORES):
        b, hg = c // 2, c % 2
        wqk_c = np.ascontiguousarray(np.concatenate(
            [w_qkv[:, hg * CQ:(hg + 1) * CQ],
             w_qkv[:, DIM + hg * CQ:DIM + (hg + 1) * CQ]], axis=1))
        wv_c = np.ascontiguousarray(w_qkv[:, 2 * DIM + hg * CQ:2 * DIM + (hg + 1) * CQ])
        wo_c = np.ascontiguousarray(w_out[hg * CQ:(hg + 1) * CQ, :])
        mv_c = np.ascontiguousarray(
            am[b].astype(np.float32).reshape(T // 128, 128).T)
        in_maps.append({
            "x": np.ascontiguousarray(x[b]),
            "wqk": wqk_c,
            "wv": wv_c,
            "wo": wo_c,
            "maskv": mv_c,
        })
    return in_maps


def run(x, w_qkv, w_out, attn_mask, trace=False):
    nc = _get_nc()
    in_maps = _make_in_maps(x, w_qkv, w_out, attn_mask)
    res = run_bass_kernel_spmd(nc, in_maps, list(range(NCORES)), trace=trace)
    outs = [res.results[c]["out"] for c in range(NCORES)]
    full = np.stack([outs[2 * b] + outs[2 * b + 1] for b in range(B)], axis=0)
    return full.astype(np.float32), res


def kernel(x, w_qkv, w_out, attn_mask):
    full, _ = run(x, w_qkv, w_out, attn_mask, trace=False)
    return full


# revision 14
# speedup vs baseline: 1.2630x; 1.2630x over previous
"""Causal multi-head attention block (qkv proj + attention + out proj) on 8
Trainium2 NeuronCores.

Sharding: core c = 2*b + hg handles batch b (of 4) and head-group hg (8 of 16
heads).  Each core computes qkv for its heads, causal attention, and a partial
out-projection (its 512 rows of w_out); the host sums the two head-group
partials per batch.

Per-core layout (all matmuls fp32r):
  - x [T, DIM] is PE-transposed into xT [DIM, T] per t-quarter; Q^T/K^T come
    out of the projection as head-pair tiles [128 = 2 heads x 64, t]; V in
    natural [t, c] layout augmented with a ones column per head (V_aug), so
    P @ V_aug accumulates the numerator and the softmax denominator together
    (softmax runs without max-subtraction: scores ~ N(0,1), exp safe in fp32).
  - scores are computed transposed, S^T[k, q], two heads concurrently via PE
    row tiling (K=64 each) into one two-bank PSUM tile; exp (scale fused) is
    one ACT op per pair, narrowed on diagonal blocks; causal masking is a 0/1
    multiply on one 128-wide strip + a zero-fill left of it.
  - normalization: denominators DMA into an [8, 512] tile, one batched DVE
    reciprocal per q-block, DMA partition-broadcast, in-place DVE multiply.
  - emission interleaves qkv quarter q+1 and out_proj q-1 (dense PE work)
    into attention block q (ACT-bound) to keep the PE busy and HAM-warm.
"""

import sys

if "/opt/trn_rl_repo" not in sys.path:
    sys.path.insert(0, "/opt/trn_rl_repo")

import numpy as np

import concourse.bass as bass
import concourse.mybir as mybir
import concourse.tile as tile
from concourse import bacc
from concourse.masks import make_identity
from concourse.bass_utils import run_bass_kernel_spmd

DIM = 1024
N_HEAD = 16
HD = 64
B, T = 4, 2048
HG = 8          # heads per core
CQ = HG * HD    # 512 feature columns per group
NCORES = 8
NT = T // 128   # 16 t-subtiles
NQ = T // 512   # 4 quarters / q-blocks

f32 = mybir.dt.float32
f32r = mybir.dt.float32r
Exp = mybir.ActivationFunctionType.Exp


def build_nc():
    nc = bacc.Bacc(None, target_bir_lowering=False)
    x_d = nc.declare_dram_parameter("x", [T, DIM], f32, isOutput=False)
    wqk_d = nc.declare_dram_parameter("wqk", [DIM, 2 * CQ], f32, isOutput=False)
    wv_d = nc.declare_dram_parameter("wv", [DIM, CQ], f32, isOutput=False)
    wo_d = nc.declare_dram_parameter("wo", [CQ, DIM], f32, isOutput=False)
    mv_d = nc.declare_dram_parameter("maskv", [128, NT], f32, isOutput=False)
    out_d = nc.declare_dram_parameter("out", [T, DIM], f32, isOutput=True)

    with tile.TileContext(nc) as tc:
        with tc.tile_pool(name="pp", bufs=1) as pp, \
             tc.tile_pool(name="qtp", bufs=2) as qtp, \
             tc.tile_pool(name="xrow", bufs=2) as xrow_p, \
             tc.tile_pool(name="xT", bufs=1) as xT_p, \
             tc.tile_pool(name="p_p", bufs=3) as p_p, \
             tc.tile_pool(name="at_p", bufs=1) as at_p, \
             tc.tile_pool(name="den_p", bufs=2) as den_p, \
             tc.tile_pool(name="rec_p", bufs=1) as rec_p, \
             tc.tile_pool(name="bcs_p", bufs=2) as bcs_p, \
             tc.tile_pool(name="out_p", bufs=2) as out_p, \
             tc.tile_pool(name="dram_p", bufs=2, space="DRAM") as dram_p, \
             tc.tile_pool(name="ps_aux", bufs=2, space="PSUM") as ps_aux, \
             tc.tile_pool(name="ps_s", bufs=1, space="PSUM") as ps_s, \
             tc.tile_pool(name="ps_pv", bufs=2, space="PSUM") as ps_pv:

            # ---- constants ----
            ident32 = pp.tile([128, 128], f32, name="ident32", tag="ident32")
            make_identity(nc, ident32)
            ident = pp.tile([128, 128], f32r, name="ident", tag="ident")
            nc.vector.tensor_copy(ident, ident32)
            # one 128x128 causal strip: keep where q_local >= k_local
            dstrip = pp.tile([128, 128], f32, name="dstrip", tag="dstrip")
            nc.gpsimd.memset(dstrip, 1.0)
            nc.gpsimd.affine_select(
                out=dstrip, in_=dstrip, compare_op=mybir.AluOpType.is_ge,
                fill=0.0, base=0, pattern=[[1, 128]], channel_multiplier=-1)
            zerosr = pp.tile([128, 384], f32r, name="zerosr", tag="zerosr")
            nc.vector.memset(zerosr.bitcast(f32), 0.0)
            onescol = pp.tile([128, HG], f32, name="onescol", tag="onescol")
            nc.vector.memset(onescol, 1.0)
            mv_sb = pp.tile([128, NT], f32, name="maskv_sb", tag="maskv_sb")
            nc.sync.dma_start(out=mv_sb, in_=mv_d[:, :])

            # ---- persistent tensors ----
            kt = [pp.tile([128, T], f32r, name=f"kt{m}", tag=f"kt{m}") for m in range(4)]
            vaug = [pp.tile([128, HG * 65], f32r, name=f"vaug{t}", tag=f"vaug{t}")
                    for t in range(NT)]
            wo_sb = [pp.tile([128, DIM], f32r, name=f"wo{m}", tag=f"wo{m}")
                     for m in range(4)]
            wqk_sb = [pp.tile([128, 2 * CQ], f32r, name=f"wqk{k}", tag=f"wqk{k}")
                      for k in range(8)]
            wv_sb = [pp.tile([128, CQ], f32r, name=f"wv{k}", tag=f"wv{k}")
                     for k in range(8)]
            for m in range(4):
                nc.sync.dma_start(out=wo_sb[m],
                                  in_=wo_d[m * 128:(m + 1) * 128, :].bitcast(f32r))
            for k in range(8):
                nc.sync.dma_start(out=wqk_sb[k],
                                  in_=wqk_d[k * 128:(k + 1) * 128, :].bitcast(f32r))
                nc.sync.dma_start(out=wv_sb[k],
                                  in_=wv_d[k * 128:(k + 1) * 128, :].bitcast(f32r))

            qt_cur = {}    # quarter -> [4 pair tiles [128, 512]]
            ats_cur = {}   # qb -> [4 pair tiles [128, 512]]

            # ---------- qkv quarter units (each closure ~1-2 us of PE) ----------
            def qkv_units(q):
                units = []
                xts = [xT_p.tile([128, 512], f32r, name=f"xt{kb}", tag=f"xt{kb}")
                       for kb in range(8)]

                def xt_unit(ti):
                    # load 128 rows of x, PE-transpose into the 8 xT tiles
                    xr = xrow_p.tile([128, DIM], f32r, name="xr", tag="xr")
                    t0 = (q * 4 + ti) * 128
                    nc.sync.dma_start(out=xr, in_=x_d[t0:t0 + 128, :].bitcast(f32r))
                    for kb in range(8):
                        pst = ps_aux.tile([128, 128], f32r, name="pst", tag="aux")
                        nc.tensor.transpose(
                            pst, xr[:, kb * 128:(kb + 1) * 128], ident)
                        nc.vector.tensor_copy(
                            xts[kb][:, ti * 128:(ti + 1) * 128], pst)
                for ti in range(4):
                    units.append(lambda ti=ti: xt_unit(ti))

                qt_cur[q] = [None] * 4

                def qk_unit(m):
                    pq = ps_aux.tile([128, 512], f32, name="mm", tag="aux")
                    for kb in range(8):
                        nc.tensor.matmul(
                            pq, wqk_sb[kb][:, m * 128:(m + 1) * 128], xts[kb],
                            start=(kb == 0), stop=(kb == 7))
                    if m < 4:
                        qtile = qtp.tile([128, 512], f32r, name=f"qt{m}", tag=f"qt{m}")
                        nc.vector.tensor_copy(qtile, pq)
                        qt_cur[q][m] = qtile
                    else:
                        nc.vector.tensor_copy(
                            kt[m - 4][:, q * 512:(q + 1) * 512], pq)
                for m in range(8):
                    units.append(lambda m=m: qk_unit(m))

                def v_unit(ti):
                    pv = ps_aux.tile([128, 512], f32, name="mm", tag="aux")
                    for kb in range(8):
                        nc.tensor.matmul(
                            pv, xts[kb][:, ti * 128:(ti + 1) * 128], wv_sb[kb],
                            start=(kb == 0), stop=(kb == 7))
                    vt = vaug[q * 4 + ti]
                    vt3 = vt.rearrange("p (h w) -> p h w", w=65)
                    nc.vector.tensor_copy(
                        vt3[:, :, 0:64], pv.rearrange("p (h w) -> p h w", w=64))
                    nc.vector.tensor_copy(
                        vt3[:, :, 64:65], onescol.rearrange("p (h w) -> p h w", w=1))
                    nc.vector.tensor_scalar_mul(
                        vt, vt, mv_sb[:, (q * 4 + ti):(q * 4 + ti + 1)])
                for ti in range(4):
                    units.append(lambda ti=ti: v_unit(ti))
                return units

            # ---------- out_proj units for one q-block ----------
            def outproj_units(qb):
                units = []

                def op_unit(ti, nb):
                    ats = ats_cur[qb]
                    po = ps_aux.tile([128, 512], f32, name="mm", tag="aux")
                    for m in range(4):
                        nc.tensor.matmul(
                            po, ats[m][:, ti * 128:(ti + 1) * 128],
                            wo_sb[m][:, nb * 512:(nb + 1) * 512],
                            start=(m == 0), stop=(m == 3))
                    ob = out_p.tile([128, 512], f32, name="ob", tag="ob")
                    nc.vector.tensor_copy(ob, po)
                    t0 = (qb * 4 + ti) * 128
                    nc.sync.dma_start(
                        out=out_d[t0:t0 + 128, nb * 512:(nb + 1) * 512], in_=ob)
                for ti in range(4):
                    for nb in range(2):
                        units.append(lambda ti=ti, nb=nb: op_unit(ti, nb))
                return units

            # ---------- attention for one q-block, interleaved with fillers ----
            def att_qb(qb, fillers):
                nk = 4 * (qb + 1)
                n_att = 4 * nk
                nf = len(fillers)
                fi = 0
                ai = 0
                ats_cur[qb] = [None] * 4
                d1 = dram_p.tile([8, 512], f32, name="d1", tag="d1")

                def pump():
                    nonlocal fi
                    while fi * n_att < ai * nf and fi < nf:
                        fillers[fi]()
                        fi += 1

                for m in range(4):
                    pvp = ps_pv.tile([65, 1024], f32, name="pv", tag="pv")
                    atm = at_p.tile([128, 512], f32r, name=f"at{m}", tag=f"at{m}")
                    ats_cur[qb][m] = atm
                    prev = None
                    for kti in range(nk):
                        sp = ps_s.tile([128, 1024], f32, name="s", tag="s")
                        nc.tensor.matmul(
                            sp[:, 0:512],
                            kt[m][0:64, kti * 128:(kti + 1) * 128],
                            qt_cur[qb][m][0:64, :], start=True, stop=True)
                        nc.tensor.matmul(
                            sp[:, 512:1024],
                            kt[m][64:128, kti * 128:(kti + 1) * 128],
                            qt_cur[qb][m][64:128, :], start=True, stop=True)
                        ppt = p_p.tile([128, 1024], f32r, name="p", tag="p")
                        p3 = ppt.rearrange("p (h w) -> p h w", w=512)
                        s3 = sp.rearrange("p (h w) -> p h w", w=512)
                        j = kti - 4 * qb
                        if j < 0:
                            nc.scalar.activation(p3, s3, Exp, scale=0.125)
                        else:
                            w0 = 128 * j
                            nc.scalar.activation(
                                p3[:, :, w0:512], s3[:, :, w0:512], Exp, scale=0.125)
                            for h in range(2):
                                if j > 0:
                                    nc.vector.tensor_copy(
                                        ppt[:, h * 512:h * 512 + w0],
                                        zerosr[:, 0:w0])
                                nc.vector.tensor_mul(
                                    ppt[:, h * 512 + w0:h * 512 + w0 + 128],
                                    ppt[:, h * 512 + w0:h * 512 + w0 + 128],
                                    dstrip)
                        if prev is not None:
                            pk, pt = prev
                            nc.tensor.matmul(
                                pvp[:, 0:512],
                                vaug[pk][:, (2 * m) * 65:(2 * m + 1) * 65],
                                pt[:, 0:512],
                                start=(pk == 0), stop=(pk == nk - 1))
                            nc.tensor.matmul(
                                pvp[:, 512:1024],
                                vaug[pk][:, (2 * m + 1) * 65:(2 * m + 2) * 65],
                                pt[:, 512:1024],
                                start=(pk == 0), stop=(pk == nk - 1))
                        prev = (kti, ppt)
                        ai += 1
                        pump()
                    pk, pt = prev
                    nc.tensor.matmul(
                        pvp[:, 0:512],
                        vaug[pk][:, (2 * m) * 65:(2 * m + 1) * 65],
                        pt[:, 0:512], start=(pk == 0), stop=True)
                    nc.tensor.matmul(
                        pvp[:, 512:1024],
                        vaug[pk][:, (2 * m + 1) * 65:(2 * m + 2) * 65],
                        pt[:, 512:1024], start=(pk == 0), stop=True)
                    # denominators out, unnormalized numerators out
                    for h in range(2):
                        dn = den_p.tile([1, 512], f32, name="dn", tag="dn")
                        nc.scalar.copy(dn, pvp[64:65, h * 512:(h + 1) * 512])
                        nc.sync.dma_start(
                            out=d1[2 * m + h:2 * m + h + 1, :], in_=dn)
                    nc.vector.tensor_copy(atm[0:64, :], pvp[0:64, 0:512])
                    nc.vector.tensor_copy(atm[64:128, :], pvp[0:64, 512:1024])
                # batched reciprocal + broadcast + in-place normalize
                # reshape the 4096 denominators to [128, 32] via DRAM so the
                # reciprocal runs 32 elems/lane instead of 512 on one lane
                den128 = rec_p.tile([128, 32], f32, name="den128", tag="den128")
                nc.sync.dma_start(
                    out=den128,
                    in_=d1.rearrange("i w -> (i w)").rearrange("(p c) -> p c", c=32))
                rec128 = rec_p.tile([128, 32], f32, name="rec128", tag="rec128")
                nc.vector.reciprocal(rec128, den128)
                d2 = dram_p.tile([8, 512], f32, name="d2", tag="d2")
                nc.sync.dma_start(
                    out=d2.rearrange("i w -> (i w)").rearrange("(p c) -> p c", c=32),
                    in_=rec128)
                for m in range(4):
                    bcs = bcs_p.tile([128, 512], f32, name="bcs", tag="bcs")
                    for h in range(2):
                        nc.sync.dma_start(
                            out=bcs[h * 64:(h + 1) * 64, :],
                            in_=d2[2 * m + h:2 * m + h + 1, :].partition_broadcast(64))
                    nc.vector.tensor_mul(ats_cur[qb][m], ats_cur[qb][m], bcs)
                while fi < nf:
                    fillers[fi]()
                    fi += 1

            # ---------------- emission schedule ----------------
            for u in qkv_units(0):
                u()
            for qb in range(NQ):
                fillers = []
                if qb - 1 >= 0:
                    fillers += outproj_units(qb - 1)
                if qb + 1 < NQ:
                    fillers += qkv_units(qb + 1)
                att_qb(qb, fillers)
            for u in outproj_units(NQ - 1):
                u()
    nc.finalize()
    return nc


_NC_CACHE = {}


def _get_nc():
    if "nc" not in _NC_CACHE:
        _NC_CACHE["nc"] = build_nc()
    return _NC_CACHE["nc"]


def _make_in_maps(x, w_qkv, w_out, attn_mask):
    x = np.asarray(x, dtype=np.float32)
    w_qkv = np.asarray(w_qkv, dtype=np.float32)
    w_out = np.asarray(w_out, dtype=np.float32)
    am = np.asarray(attn_mask)
    in_maps = []
    for c in range(NCORES):
        b, hg = c // 2, c % 2
        wqk_c = np.ascontiguousarray(np.concatenate(
            [w_qkv[:, hg * CQ:(hg + 1) * CQ],
             w_qkv[:, DIM + hg * CQ:DIM + (hg + 1) * CQ]], axis=1))
        wv_c = np.ascontiguousarray(w_qkv[:, 2 * DIM + hg * CQ:2 * DIM + (hg + 1) * CQ])
        wo_c = np.ascontiguousarray(w_out[hg * CQ:(hg + 1) * CQ, :])
        mv_c = np.ascontiguousarray(
            am[b].astype(np.float32).reshape(NT, 128).T)
        in_maps.append({
            "x": np.ascontiguousarray(x[b]),
            "wqk": wqk_c,
            "wv": wv_c,
            "wo": wo_c,
            "maskv": mv_c,
        })
    return in_maps


def run(x, w_qkv, w_out, attn_mask, trace=False):
    nc = _get_nc()
    in_maps = _make_in_maps(x, w_qkv, w_out, attn_mask)
    res = run_bass_kernel_spmd(nc, in_maps, list(range(NCORES)), trace=trace)
    outs = [res.results[c]["out"] for c in range(NCORES)]
    full = np.stack([outs[2 * b] + outs[2 * b + 1] for b in range(B)], axis=0)
    return full.astype(np.float32), res


def kernel(x, w_qkv, w_out, attn_mask):
    full, _ = run(x, w_qkv, w_out, attn_mask, trace=False)
    return full


# revision 15
# speedup vs baseline: 1.3899x; 1.1005x over previous
"""Causal multi-head attention block (qkv proj + attention + out proj) on 8
Trainium2 NeuronCores.

Sharding: core c = 2*b + hg handles batch b (of 4) and head-group hg (8 of 16
heads).  Each core computes qkv for its heads, causal attention, and a partial
out-projection (its 512 rows of w_out); the host sums the two head-group
partials per batch.

Per-core layout (all matmuls fp32r):
  - x [T, DIM] is PE-transposed into xT [DIM, T] per t-quarter; Q^T/K^T come
    out of the projection as head-pair tiles [128 = 2 heads x 64, t]; V in
    natural [t, c] layout augmented with a ones column per head (V_aug), so
    P @ V_aug accumulates the numerator and the softmax denominator together
    (softmax runs without max-subtraction: scores ~ N(0,1), exp safe in fp32).
  - scores are computed transposed, S^T[k, q], two heads concurrently via PE
    row tiling (K=64 each) into one two-bank PSUM tile; exp (scale fused) is
    one ACT op per pair, narrowed on diagonal blocks; causal masking is a 0/1
    multiply on one 128-wide strip + a zero-fill left of it.
  - normalization: denominators DMA into an [8, 512] tile, one batched DVE
    reciprocal per q-block, DMA partition-broadcast, in-place DVE multiply.
  - emission interleaves qkv quarter q+1 and out_proj q-1 (dense PE work)
    into attention block q (ACT-bound) to keep the PE busy and HAM-warm.
"""

import sys

if "/opt/trn_rl_repo" not in sys.path:
    sys.path.insert(0, "/opt/trn_rl_repo")

import numpy as np

import concourse.bass as bass
import concourse.mybir as mybir
import concourse.tile as tile
from concourse import bacc
from concourse.masks import make_identity
from concourse.bass_utils import run_bass_kernel_spmd

DIM = 1024
N_HEAD = 16
HD = 64
B, T = 4, 2048
HG = 8          # heads per core
CQ = HG * HD    # 512 feature columns per group
NCORES = 8
NT = T // 128   # 16 t-subtiles
NQ = T // 512   # 4 quarters / q-blocks

f32 = mybir.dt.float32
f32r = mybir.dt.float32r
Exp = mybir.ActivationFunctionType.Exp


def build_nc():
    nc = bacc.Bacc(None, target_bir_lowering=False)
    x_d = nc.declare_dram_parameter("x", [T, DIM], f32, isOutput=False)
    wqk_d = nc.declare_dram_parameter("wqk", [DIM, 2 * CQ], f32, isOutput=False)
    wv_d = nc.declare_dram_parameter("wv", [DIM, CQ], f32, isOutput=False)
    wo_d = nc.declare_dram_parameter("wo", [CQ, DIM], f32, isOutput=False)
    mv_d = nc.declare_dram_parameter("maskv", [128, NT], f32, isOutput=False)
    out_d = nc.declare_dram_parameter("out", [T, DIM], f32, isOutput=True)

    with tile.TileContext(nc) as tc:
        with tc.tile_pool(name="pp", bufs=1) as pp, \
             tc.tile_pool(name="qtp", bufs=2) as qtp, \
             tc.tile_pool(name="xrow", bufs=2) as xrow_p, \
             tc.tile_pool(name="xT", bufs=1) as xT_p, \
             tc.tile_pool(name="p_p", bufs=3) as p_p, \
             tc.tile_pool(name="at_p", bufs=1) as at_p, \
             tc.tile_pool(name="den_p", bufs=2) as den_p, \
             tc.tile_pool(name="rec_p", bufs=1) as rec_p, \
             tc.tile_pool(name="bcs_p", bufs=2) as bcs_p, \
             tc.tile_pool(name="out_p", bufs=2) as out_p, \
             tc.tile_pool(name="dram_p", bufs=2, space="DRAM") as dram_p, \
             tc.tile_pool(name="ps_aux", bufs=2, space="PSUM") as ps_aux, \
             tc.tile_pool(name="ps_s", bufs=2, space="PSUM") as ps_s, \
             tc.tile_pool(name="ps_pv", bufs=1, space="PSUM") as ps_pv:

            # ---- constants ----
            ident32 = pp.tile([128, 128], f32, name="ident32", tag="ident32")
            make_identity(nc, ident32)
            ident = pp.tile([128, 128], f32r, name="ident", tag="ident")
            nc.vector.tensor_copy(ident, ident32)
            # one 128x128 causal strip: keep where q_local >= k_local
            dstrip = pp.tile([128, 128], f32, name="dstrip", tag="dstrip")
            nc.gpsimd.memset(dstrip, 1.0)
            nc.gpsimd.affine_select(
                out=dstrip, in_=dstrip, compare_op=mybir.AluOpType.is_ge,
                fill=0.0, base=0, pattern=[[1, 128]], channel_multiplier=-1)
            zerosr = pp.tile([128, 384], f32r, name="zerosr", tag="zerosr")
            nc.vector.memset(zerosr.bitcast(f32), 0.0)
            onescol = pp.tile([128, HG], f32, name="onescol", tag="onescol")
            nc.vector.memset(onescol, 1.0)
            mv_sb = pp.tile([128, NT], f32, name="maskv_sb", tag="maskv_sb")
            nc.sync.dma_start(out=mv_sb, in_=mv_d[:, :])

            # ---- persistent tensors ----
            kt = [pp.tile([128, T], f32r, name=f"kt{m}", tag=f"kt{m}") for m in range(4)]
            vaug = [pp.tile([128, HG * 65], f32r, name=f"vaug{t}", tag=f"vaug{t}")
                    for t in range(NT)]
            wo_sb = [pp.tile([128, DIM], f32r, name=f"wo{m}", tag=f"wo{m}")
                     for m in range(4)]
            wqk_sb = [pp.tile([128, 2 * CQ], f32r, name=f"wqk{k}", tag=f"wqk{k}")
                      for k in range(8)]
            wv_sb = [pp.tile([128, CQ], f32r, name=f"wv{k}", tag=f"wv{k}")
                     for k in range(8)]
            for m in range(4):
                nc.sync.dma_start(out=wo_sb[m],
                                  in_=wo_d[m * 128:(m + 1) * 128, :].bitcast(f32r))
            for k in range(8):
                nc.sync.dma_start(out=wqk_sb[k],
                                  in_=wqk_d[k * 128:(k + 1) * 128, :].bitcast(f32r))
                nc.sync.dma_start(out=wv_sb[k],
                                  in_=wv_d[k * 128:(k + 1) * 128, :].bitcast(f32r))

            qt_cur = {}    # quarter -> [4 pair tiles [128, 512]]
            ats_cur = {}   # qb -> [4 pair tiles [128, 512]]

            # ---------- qkv quarter units (each closure ~1-2 us of PE) ----------
            def qkv_units(q):
                units = []
                xts = [xT_p.tile([128, 512], f32r, name=f"xt{kb}", tag=f"xt{kb}")
                       for kb in range(8)]

                def xt_unit(ti):
                    # load 128 rows of x, PE-transpose into the 8 xT tiles
                    xr = xrow_p.tile([128, DIM], f32r, name="xr", tag="xr")
                    t0 = (q * 4 + ti) * 128
                    nc.sync.dma_start(out=xr, in_=x_d[t0:t0 + 128, :].bitcast(f32r))
                    for kb in range(8):
                        pst = ps_aux.tile([128, 128], f32r, name="pst", tag="aux")
                        nc.tensor.transpose(
                            pst, xr[:, kb * 128:(kb + 1) * 128], ident)
                        nc.vector.tensor_copy(
                            xts[kb][:, ti * 128:(ti + 1) * 128], pst)
                for ti in range(4):
                    units.append(lambda ti=ti: xt_unit(ti))

                qt_cur[q] = [None] * 4

                def qk_unit(m):
                    pq = ps_aux.tile([128, 512], f32, name="mm", tag="aux")
                    for kb in range(8):
                        nc.tensor.matmul(
                            pq, wqk_sb[kb][:, m * 128:(m + 1) * 128], xts[kb],
                            start=(kb == 0), stop=(kb == 7))
                    if m < 4:
                        qtile = qtp.tile([128, 512], f32r, name=f"qt{m}", tag=f"qt{m}")
                        nc.vector.tensor_copy(qtile, pq)
                        qt_cur[q][m] = qtile
                    else:
                        nc.vector.tensor_copy(
                            kt[m - 4][:, q * 512:(q + 1) * 512], pq)
                for m in range(8):
                    units.append(lambda m=m: qk_unit(m))

                def v_unit(ti):
                    pv = ps_aux.tile([128, 512], f32, name="mm", tag="aux")
                    for kb in range(8):
                        nc.tensor.matmul(
                            pv, xts[kb][:, ti * 128:(ti + 1) * 128], wv_sb[kb],
                            start=(kb == 0), stop=(kb == 7))
                    vt = vaug[q * 4 + ti]
                    vt3 = vt.rearrange("p (h w) -> p h w", w=65)
                    nc.vector.tensor_copy(
                        vt3[:, :, 0:64], pv.rearrange("p (h w) -> p h w", w=64))
                    nc.vector.tensor_copy(
                        vt3[:, :, 64:65], onescol.rearrange("p (h w) -> p h w", w=1))
                    nc.vector.tensor_scalar_mul(
                        vt, vt, mv_sb[:, (q * 4 + ti):(q * 4 + ti + 1)])
                for ti in range(4):
                    units.append(lambda ti=ti: v_unit(ti))
                return units

            # ---------- out_proj units for one q-block ----------
            def outproj_units(qb):
                units = []

                def op_unit(ti, nb):
                    ats = ats_cur[qb]
                    po = ps_aux.tile([128, 512], f32, name="mm", tag="aux")
                    for m in range(4):
                        nc.tensor.matmul(
                            po, ats[m][:, ti * 128:(ti + 1) * 128],
                            wo_sb[m][:, nb * 512:(nb + 1) * 512],
                            start=(m == 0), stop=(m == 3))
                    ob = out_p.tile([128, 512], f32, name="ob", tag="ob")
                    nc.vector.tensor_copy(ob, po)
                    t0 = (qb * 4 + ti) * 128
                    nc.sync.dma_start(
                        out=out_d[t0:t0 + 128, nb * 512:(nb + 1) * 512], in_=ob)
                for ti in range(4):
                    for nb in range(2):
                        units.append(lambda ti=ti, nb=nb: op_unit(ti, nb))
                return units

            # ---------- attention for one q-block, interleaved with fillers ----
            def att_qb(qb, fillers):
                nk = 4 * (qb + 1)
                n_att = 4 * nk
                nf = len(fillers)
                fi = 0
                ai = 0
                ats_cur[qb] = [None] * 4
                d1 = dram_p.tile([8, 512], f32, name="d1", tag="d1")

                def pump():
                    nonlocal fi
                    while fi * n_att < ai * nf and fi < nf:
                        fillers[fi]()
                        fi += 1

                for m in range(4):
                    pvp = ps_pv.tile([65, 1024], f32, name="pv", tag="pv")
                    atm = at_p.tile([128, 512], f32r, name=f"at{m}", tag=f"at{m}")
                    ats_cur[qb][m] = atm
                    prev = None
                    for kti in range(nk):
                        sp = ps_s.tile([128, 1024], f32, name="s", tag="s")
                        nc.tensor.matmul(
                            sp[:, 0:512],
                            kt[m][0:64, kti * 128:(kti + 1) * 128],
                            qt_cur[qb][m][0:64, :], start=True, stop=True)
                        nc.tensor.matmul(
                            sp[:, 512:1024],
                            kt[m][64:128, kti * 128:(kti + 1) * 128],
                            qt_cur[qb][m][64:128, :], start=True, stop=True)
                        ppt = p_p.tile([128, 1024], f32r, name="p", tag="p")
                        p3 = ppt.rearrange("p (h w) -> p h w", w=512)
                        s3 = sp.rearrange("p (h w) -> p h w", w=512)
                        j = kti - 4 * qb
                        if j < 0:
                            nc.scalar.activation(p3, s3, Exp, scale=0.125)
                        else:
                            w0 = 128 * j
                            nc.scalar.activation(
                                p3[:, :, w0:512], s3[:, :, w0:512], Exp, scale=0.125)
                            for h in range(2):
                                if j > 0:
                                    nc.vector.tensor_copy(
                                        ppt[:, h * 512:h * 512 + w0],
                                        zerosr[:, 0:w0])
                                nc.vector.tensor_mul(
                                    ppt[:, h * 512 + w0:h * 512 + w0 + 128],
                                    ppt[:, h * 512 + w0:h * 512 + w0 + 128],
                                    dstrip)
                        if prev is not None:
                            pk, pt = prev
                            nc.tensor.matmul(
                                pvp[:, 0:512],
                                vaug[pk][:, (2 * m) * 65:(2 * m + 1) * 65],
                                pt[:, 0:512],
                                start=(pk == 0), stop=(pk == nk - 1))
                            nc.tensor.matmul(
                                pvp[:, 512:1024],
                                vaug[pk][:, (2 * m + 1) * 65:(2 * m + 2) * 65],
                                pt[:, 512:1024],
                                start=(pk == 0), stop=(pk == nk - 1))
                        prev = (kti, ppt)
                        ai += 1
                        pump()
                    pk, pt = prev
                    nc.tensor.matmul(
                        pvp[:, 0:512],
                        vaug[pk][:, (2 * m) * 65:(2 * m + 1) * 65],
                        pt[:, 0:512], start=(pk == 0), stop=True)
                    nc.tensor.matmul(
                        pvp[:, 512:1024],
                        vaug[pk][:, (2 * m + 1) * 65:(2 * m + 2) * 65],
                        pt[:, 512:1024], start=(pk == 0), stop=True)
                    # denominators out, unnormalized numerators out
                    for h in range(2):
                        dn = den_p.tile([1, 512], f32, name="dn", tag="dn")
                        nc.vector.tensor_copy(dn, pvp[64:65, h * 512:(h + 1) * 512])
                        nc.sync.dma_start(
                            out=d1[2 * m + h:2 * m + h + 1, :], in_=dn)
                    nc.vector.tensor_copy(atm[0:64, :], pvp[0:64, 0:512])
                    nc.vector.tensor_copy(atm[64:128, :], pvp[0:64, 512:1024])
                # batched reciprocal + broadcast + in-place normalize
                # reshape the 4096 denominators to [128, 32] via DRAM so the
                # reciprocal runs 32 elems/lane instead of 512 on one lane
                den128 = rec_p.tile([128, 32], f32, name="den128", tag="den128")
                nc.sync.dma_start(
                    out=den128,
                    in_=d1.rearrange("i w -> (i w)").rearrange("(p c) -> p c", c=32))
                rec128 = rec_p.tile([128, 32], f32, name="rec128", tag="rec128")
                nc.vector.reciprocal(rec128, den128)
                d2 = dram_p.tile([8, 512], f32, name="d2", tag="d2")
                nc.sync.dma_start(
                    out=d2.rearrange("i w -> (i w)").rearrange("(p c) -> p c", c=32),
                    in_=rec128)
                for m in range(4):
                    bcs = bcs_p.tile([128, 512], f32, name="bcs", tag="bcs")
                    for h in range(2):
                        nc.sync.dma_start(
                            out=bcs[h * 64:(h + 1) * 64, :],
                            in_=d2[2 * m + h:2 * m + h + 1, :].partition_broadcast(64))
                    nc.vector.tensor_mul(ats_cur[qb][m], ats_cur[qb][m], bcs)
                while fi < nf:
                    fillers[fi]()
                    fi += 1

            # ---------------- emission schedule ----------------
            for u in qkv_units(0):
                u()
            for qb in range(NQ):
                fillers = []
                if qb - 1 >= 0:
                    fillers += outproj_units(qb - 1)
                if qb + 1 < NQ:
                    fillers += qkv_units(qb + 1)
                att_qb(qb, fillers)
            for u in outproj_units(NQ - 1):
                u()
    nc.finalize()
    return nc


_NC_CACHE = {}


def _get_nc():
    if "nc" not in _NC_CACHE:
        _NC_CACHE["nc"] = build_nc()
    return _NC_CACHE["nc"]


def _make_in_maps(x, w_qkv, w_out, attn_mask):
    x = np.asarray(x, dtype=np.float32)
    w_qkv = np.asarray(w_qkv, dtype=np.float32)
    w_out = np.asarray(w_out, dtype=np.float32)
    am = np.asarray(attn_mask)
    in_maps = []
    for c in range(NCORES):
        b, hg = c // 2, c % 2
        wqk_c = np.ascontiguousarray(np.concatenate(
            [w_qkv[:, hg * CQ:(hg + 1) * CQ],
             w_qkv[:, DIM + hg * CQ:DIM + (hg + 1) * CQ]], axis=1))
        wv_c = np.ascontiguousarray(w_qkv[:, 2 * DIM + hg * CQ:2 * DIM + (hg + 1) * CQ])
        wo_c = np.ascontiguousarray(w_out[hg * CQ:(hg + 1) * CQ, :])
        mv_c = np.ascontiguousarray(
            am[b].astype(np.float32).reshape(NT, 128).T)
        in_maps.append({
            "x": np.ascontiguousarray(x[b]),
            "wqk": wqk_c,
            "wv": wv_c,
            "wo": wo_c,
            "maskv": mv_c,
        })
    return in_maps


def run(x, w_qkv, w_out, attn_mask, trace=False):
    nc = _get_nc()
    in_maps = _make_in_maps(x, w_qkv, w_out, attn_mask)
    res = run_bass_kernel_spmd(nc, in_maps, list(range(NCORES)), trace=trace)
    outs = [res.results[c]["out"] for c in range(NCORES)]
    full = np.stack([outs[2 * b] + outs[2 * b + 1] for b in range(B)], axis=0)
    return full.astype(np.float32), res


def kernel(x, w_qkv, w_out, attn_mask):
    full, _ = run(x, w_qkv, w_out, attn_mask, trace=False)
    return full


# revision 17
# speedup vs baseline: 1.4230x; 1.0238x over previous
"""Causal multi-head attention block (qkv proj + attention + out proj) on 8
Trainium2 NeuronCores.

Sharding: core c = 2*b + hg handles batch b (of 4) and head-group hg (8 of 16
heads).  Each core computes qkv for its heads, causal attention, and a partial
out-projection (its 512 rows of w_out); the host sums the two head-group
partials per batch.

Per-core layout (all matmuls fp32r):
  - x [T, DIM] is PE-transposed into xT [DIM, T] per t-quarter; Q^T/K^T come
    out of the projection as head-pair tiles [128 = 2 heads x 64, t]; V in
    natural [t, c] layout augmented with a ones column per head (V_aug), so
    P @ V_aug accumulates the numerator and the softmax denominator together
    (softmax runs without max-subtraction: scores ~ N(0,1), exp safe in fp32).
  - scores are computed transposed, S^T[k, q], two heads concurrently via PE
    row tiling (K=64 each) into one two-bank PSUM tile; exp (scale fused) is
    one ACT op per pair, narrowed on diagonal blocks; causal masking is a 0/1
    multiply on one 128-wide strip + a zero-fill left of it.
  - normalization: denominators DMA into an [8, 512] tile, one batched DVE
    reciprocal per q-block, DMA partition-broadcast, in-place DVE multiply.
  - emission interleaves qkv quarter q+1 and out_proj q-1 (dense PE work)
    into attention block q (ACT-bound) to keep the PE busy and HAM-warm.
"""

import sys

if "/opt/trn_rl_repo" not in sys.path:
    sys.path.insert(0, "/opt/trn_rl_repo")

import numpy as np

import concourse.bass as bass
import concourse.mybir as mybir
import concourse.tile as tile
from concourse import bacc
from concourse.masks import make_identity
from concourse.bass_utils import run_bass_kernel_spmd

DIM = 1024
N_HEAD = 16
HD = 64
B, T = 4, 2048
HG = 8          # heads per core
CQ = HG * HD    # 512 feature columns per group
NCORES = 8
NT = T // 128   # 16 t-subtiles
NQ = T // 512   # 4 quarters / q-blocks

f32 = mybir.dt.float32
f32r = mybir.dt.float32r
Exp = mybir.ActivationFunctionType.Exp


def build_nc():
    nc = bacc.Bacc(None, target_bir_lowering=False)
    x_d = nc.declare_dram_parameter("x", [T, DIM], f32, isOutput=False)
    wqk_d = nc.declare_dram_parameter("wqk", [DIM, 2 * CQ], f32, isOutput=False)
    wv_d = nc.declare_dram_parameter("wv", [DIM, CQ], f32, isOutput=False)
    wo_d = nc.declare_dram_parameter("wo", [CQ, DIM], f32, isOutput=False)
    mv_d = nc.declare_dram_parameter("maskv", [128, NT], f32, isOutput=False)
    out_d = nc.declare_dram_parameter("out", [T, DIM], f32, isOutput=True)

    with tile.TileContext(nc) as tc:
        with tc.tile_pool(name="pp", bufs=1) as pp, \
             tc.tile_pool(name="qtp", bufs=2) as qtp, \
             tc.tile_pool(name="xrow", bufs=2) as xrow_p, \
             tc.tile_pool(name="xT", bufs=1) as xT_p, \
             tc.tile_pool(name="p_p", bufs=3) as p_p, \
             tc.tile_pool(name="at_p", bufs=1) as at_p, \
             tc.tile_pool(name="den_p", bufs=2) as den_p, \
             tc.tile_pool(name="rec_p", bufs=1) as rec_p, \
             tc.tile_pool(name="bcs_p", bufs=2) as bcs_p, \
             tc.tile_pool(name="out_p", bufs=2) as out_p, \
             tc.tile_pool(name="dram_p", bufs=2, space="DRAM") as dram_p, \
             tc.tile_pool(name="ps_aux", bufs=2, space="PSUM") as ps_aux, \
             tc.tile_pool(name="ps_s", bufs=2, space="PSUM") as ps_s, \
             tc.tile_pool(name="ps_pv", bufs=1, space="PSUM") as ps_pv:

            # ---- constants ----
            ident32 = pp.tile([128, 128], f32, name="ident32", tag="ident32")
            make_identity(nc, ident32)
            ident = pp.tile([128, 128], f32r, name="ident", tag="ident")
            nc.vector.tensor_copy(ident, ident32)
            # one 128x128 causal strip: keep where q_local >= k_local
            dstrip = pp.tile([128, 128], f32, name="dstrip", tag="dstrip")
            nc.gpsimd.memset(dstrip, 1.0)
            nc.gpsimd.affine_select(
                out=dstrip, in_=dstrip, compare_op=mybir.AluOpType.is_ge,
                fill=0.0, base=0, pattern=[[1, 128]], channel_multiplier=-1)
            zerosr = pp.tile([128, 384], f32r, name="zerosr", tag="zerosr")
            nc.vector.memset(zerosr.bitcast(f32), 0.0)
            onescol = pp.tile([128, HG], f32, name="onescol", tag="onescol")
            nc.vector.memset(onescol, 1.0)
            mv_sb = pp.tile([128, NT], f32, name="maskv_sb", tag="maskv_sb")
            nc.sync.dma_start(out=mv_sb, in_=mv_d[:, :])

            # ---- persistent tensors ----
            kt = [pp.tile([128, T], f32r, name=f"kt{m}", tag=f"kt{m}") for m in range(4)]
            vaug = [pp.tile([128, HG * 65], f32r, name=f"vaug{t}", tag=f"vaug{t}")
                    for t in range(NT)]
            wo_sb = [pp.tile([128, DIM], f32r, name=f"wo{m}", tag=f"wo{m}")
                     for m in range(4)]
            wqk_sb = [pp.tile([128, 2 * CQ], f32r, name=f"wqk{k}", tag=f"wqk{k}")
                      for k in range(8)]
            wv_sb = [pp.tile([128, CQ], f32r, name=f"wv{k}", tag=f"wv{k}")
                     for k in range(8)]
            for m in range(4):
                nc.sync.dma_start(out=wo_sb[m],
                                  in_=wo_d[m * 128:(m + 1) * 128, :].bitcast(f32r))
            for k in range(8):
                nc.sync.dma_start(out=wqk_sb[k],
                                  in_=wqk_d[k * 128:(k + 1) * 128, :].bitcast(f32r))
                nc.sync.dma_start(out=wv_sb[k],
                                  in_=wv_d[k * 128:(k + 1) * 128, :].bitcast(f32r))

            qt_cur = {}    # quarter -> [4 pair tiles [128, 512]]
            ats_cur = {}   # qb -> [4 pair tiles [128, 512]]

            # ---------- qkv quarter units (each closure ~1-2 us of PE) ----------
            def qkv_units(q):
                units = []
                xts = [xT_p.tile([128, 512], f32r, name=f"xt{kb}", tag=f"xt{kb}")
                       for kb in range(8)]

                def xt_unit(ti):
                    # load 128 rows of x, PE-transpose into the 8 xT tiles
                    xr = xrow_p.tile([128, DIM], f32r, name="xr", tag="xr")
                    t0 = (q * 4 + ti) * 128
                    nc.sync.dma_start(out=xr, in_=x_d[t0:t0 + 128, :].bitcast(f32r))
                    for kb in range(8):
                        pst = ps_aux.tile([128, 128], f32r, name="pst", tag="aux")
                        nc.tensor.transpose(
                            pst, xr[:, kb * 128:(kb + 1) * 128], ident)
                        nc.vector.tensor_copy(
                            xts[kb][:, ti * 128:(ti + 1) * 128], pst)
                for ti in range(4):
                    units.append(lambda ti=ti: xt_unit(ti))

                qt_cur[q] = [None] * 4

                def qk_unit(m):
                    pq = ps_aux.tile([128, 512], f32, name="mm", tag="aux")
                    for kb in range(8):
                        nc.tensor.matmul(
                            pq, wqk_sb[kb][:, m * 128:(m + 1) * 128], xts[kb],
                            start=(kb == 0), stop=(kb == 7))
                    if m < 4:
                        qtile = qtp.tile([128, 512], f32r, name=f"qt{m}", tag=f"qt{m}")
                        nc.vector.tensor_copy(qtile, pq)
                        qt_cur[q][m] = qtile
                    else:
                        nc.vector.tensor_copy(
                            kt[m - 4][:, q * 512:(q + 1) * 512], pq)
                for m in range(8):
                    units.append(lambda m=m: qk_unit(m))

                def v_unit(ti):
                    pv = ps_aux.tile([128, 512], f32, name="mm", tag="aux")
                    for kb in range(8):
                        nc.tensor.matmul(
                            pv, xts[kb][:, ti * 128:(ti + 1) * 128], wv_sb[kb],
                            start=(kb == 0), stop=(kb == 7))
                    vt = vaug[q * 4 + ti]
                    vt3 = vt.rearrange("p (h w) -> p h w", w=65)
                    nc.vector.tensor_copy(
                        vt3[:, :, 0:64], pv.rearrange("p (h w) -> p h w", w=64))
                    nc.vector.tensor_copy(
                        vt3[:, :, 64:65], onescol.rearrange("p (h w) -> p h w", w=1))
                    nc.vector.tensor_scalar_mul(
                        vt, vt, mv_sb[:, (q * 4 + ti):(q * 4 + ti + 1)])
                for ti in range(4):
                    units.append(lambda ti=ti: v_unit(ti))
                return units

            # ---------- out_proj units for one q-block ----------
            def outproj_units(qb):
                units = []

                def op_unit(ti, nb):
                    ats = ats_cur[qb]
                    po = ps_aux.tile([128, 512], f32, name="mm", tag="aux")
                    for m in range(4):
                        nc.tensor.matmul(
                            po, ats[m][:, ti * 128:(ti + 1) * 128],
                            wo_sb[m][:, nb * 512:(nb + 1) * 512],
                            start=(m == 0), stop=(m == 3))
                    ob = out_p.tile([128, 512], f32, name="ob", tag="ob")
                    nc.vector.tensor_copy(ob, po)
                    t0 = (qb * 4 + ti) * 128
                    nc.sync.dma_start(
                        out=out_d[t0:t0 + 128, nb * 512:(nb + 1) * 512], in_=ob)
                for ti in range(4):
                    for nb in range(2):
                        units.append(lambda ti=ti, nb=nb: op_unit(ti, nb))
                return units

            # ---------- attention for one q-block, interleaved with fillers ----
            def att_qb(qb, fillers):
                nk = 4 * (qb + 1)
                n_att = 4 * nk
                nf = len(fillers)
                fi = 0
                ai = 0
                ats_cur[qb] = [None] * 4
                d1 = dram_p.tile([8, 512], f32, name="d1", tag="d1")

                def pump():
                    nonlocal fi
                    while fi * n_att < ai * nf and fi < nf:
                        fillers[fi]()
                        fi += 1

                for m in range(4):
                    pvp = ps_pv.tile([65, 1024], f32, name="pv", tag="pv")
                    atm = at_p.tile([128, 512], f32r, name=f"at{m}", tag=f"at{m}")
                    ats_cur[qb][m] = atm
                    prev = None
                    for kti in range(nk):
                        sp = ps_s.tile([128, 1024], f32, name="s", tag="s")
                        nc.tensor.matmul(
                            sp[:, 0:512],
                            kt[m][0:64, kti * 128:(kti + 1) * 128],
                            qt_cur[qb][m][0:64, :], start=True, stop=True)
                        nc.tensor.matmul(
                            sp[:, 512:1024],
                            kt[m][64:128, kti * 128:(kti + 1) * 128],
                            qt_cur[qb][m][64:128, :], start=True, stop=True)
                        ppt = p_p.tile([128, 1024], f32r, name="p", tag="p")
                        p3 = ppt.rearrange("p (h w) -> p h w", w=512)
                        s3 = sp.rearrange("p (h w) -> p h w", w=512)
                        j = kti - 4 * qb
                        if j < 0:
                            nc.scalar.activation(p3, s3, Exp, scale=0.125)
                        else:
                            w0 = 128 * j
                            nc.scalar.activation(
                                p3[:, :, w0:512], s3[:, :, w0:512], Exp, scale=0.125)
                            for h in range(2):
                                if j > 0:
                                    nc.vector.tensor_copy(
                                        ppt[:, h * 512:h * 512 + w0],
                                        zerosr[:, 0:w0])
                                nc.vector.tensor_mul(
                                    ppt[:, h * 512 + w0:h * 512 + w0 + 128],
                                    ppt[:, h * 512 + w0:h * 512 + w0 + 128],
                                    dstrip)
                        if prev is not None:
                            pk, pt = prev
                            nc.tensor.matmul(
                                pvp[:, 0:512],
                                vaug[pk][:, (2 * m) * 65:(2 * m + 1) * 65],
                                pt[:, 0:512],
                                start=(pk == 0), stop=(pk == nk - 1))
                            nc.tensor.matmul(
                                pvp[:, 512:1024],
                                vaug[pk][:, (2 * m + 1) * 65:(2 * m + 2) * 65],
                                pt[:, 512:1024],
                                start=(pk == 0), stop=(pk == nk - 1))
                        prev = (kti, ppt)
                        ai += 1
                        pump()
                    pk, pt = prev
                    nc.tensor.matmul(
                        pvp[:, 0:512],
                        vaug[pk][:, (2 * m) * 65:(2 * m + 1) * 65],
                        pt[:, 0:512], start=(pk == 0), stop=True)
                    nc.tensor.matmul(
                        pvp[:, 512:1024],
                        vaug[pk][:, (2 * m + 1) * 65:(2 * m + 2) * 65],
                        pt[:, 512:1024], start=(pk == 0), stop=True)
                    # denominators out, unnormalized numerators out
                    for h in range(2):
                        dn = den_p.tile([1, 512], f32, name="dn", tag="dn")
                        nc.vector.tensor_copy(dn, pvp[64:65, h * 512:(h + 1) * 512])
                        nc.sync.dma_start(
                            out=d1[2 * m + h:2 * m + h + 1, :], in_=dn)
                    nc.vector.tensor_copy(atm[0:64, :], pvp[0:64, 0:512])
                    nc.vector.tensor_copy(atm[64:128, :], pvp[0:64, 512:1024])
                # batched reciprocal + broadcast + in-place normalize
                # reshape the 4096 denominators to [128, 32] via DRAM so the
                # reciprocal runs 32 elems/lane instead of 512 on one lane
                den128 = rec_p.tile([128, 32], f32, name="den128", tag="den128")
                nc.sync.dma_start(
                    out=den128,
                    in_=d1.rearrange("i w -> (i w)").rearrange("(p c) -> p c", c=32))
                rec128 = rec_p.tile([128, 32], f32, name="rec128", tag="rec128")
                nc.vector.reciprocal(rec128, den128)
                d2 = dram_p.tile([8, 512], f32, name="d2", tag="d2")
                nc.sync.dma_start(
                    out=d2.rearrange("i w -> (i w)").rearrange("(p c) -> p c", c=32),
                    in_=rec128)
                for m in range(4):
                    bcs = bcs_p.tile([128, 512], f32, name="bcs", tag="bcs")
                    for h in range(2):
                        nc.sync.dma_start(
                            out=bcs[h * 64:(h + 1) * 64, :],
                            in_=d2[2 * m + h:2 * m + h + 1, :].partition_broadcast(64))
                    nc.vector.tensor_mul(ats_cur[qb][m], ats_cur[qb][m], bcs)
                while fi < nf:
                    fillers[fi]()
                    fi += 1

            # ---------------- emission schedule ----------------
            for u in qkv_units(0):
                u()
            for qb in range(NQ):
                fillers = []
                if qb - 1 >= 0:
                    fillers += outproj_units(qb - 1)
                if qb + 1 < NQ:
                    fillers += qkv_units(qb + 1)
                att_qb(qb, fillers)
            for u in outproj_units(NQ - 1):
                u()
    nc.finalize()
    return nc


_NC_CACHE = {}


def _get_nc():
    if "nc" not in _NC_CACHE:
        _NC_CACHE["nc"] = build_nc()
    return _NC_CACHE["nc"]


def _make_in_maps(x, w_qkv, w_out, attn_mask):
    x = np.asarray(x, dtype=np.float32)
    w_qkv = np.asarray(w_qkv, dtype=np.float32)
    w_out = np.asarray(w_out, dtype=np.float32)
    am = np.asarray(attn_mask)
    in_maps = []
    for c in range(NCORES):
        b, hg = c // 2, c % 2
        wqk_c = np.ascontiguousarray(np.concatenate(
            [w_qkv[:, hg * CQ:(hg + 1) * CQ],
             w_qkv[:, DIM + hg * CQ:DIM + (hg + 1) * CQ]], axis=1))
        wv_c = np.ascontiguousarray(w_qkv[:, 2 * DIM + hg * CQ:2 * DIM + (hg + 1) * CQ])
        wo_c = np.ascontiguousarray(w_out[hg * CQ:(hg + 1) * CQ, :])
        mv_c = np.ascontiguousarray(
            am[b].astype(np.float32).reshape(NT, 128).T)
        in_maps.append({
            "x": np.ascontiguousarray(x[b]),
            "wqk": wqk_c,
            "wv": wv_c,
            "wo": wo_c,
            "maskv": mv_c,
        })
    return in_maps


def run(x, w_qkv, w_out, attn_mask, trace=False):
    nc = _get_nc()
    in_maps = _make_in_maps(x, w_qkv, w_out, attn_mask)
    res = run_bass_kernel_spmd(nc, in_maps, list(range(NCORES)), trace=trace)
    outs = [res.results[c]["out"] for c in range(NCORES)]
    full = np.stack([outs[2 * b] + outs[2 * b + 1] for b in range(B)], axis=0)
    return full.astype(np.float32), res


def kernel(x, w_qkv, w_out, attn_mask):
    full, _ = run(x, w_qkv, w_out, attn_mask, trace=False)
    return full
